# revision 71
# baseline (speedup 1.0000x reference)
"""Sharded attention kernel for Trainium2 (8 NeuronCores, Bass/Tile).

Module: x->(wq,wk,wv) qk-norm + rope + GQA self-attn  (+)  gated cross-attn
over y->(wk_y,wv_y), then wo.  B=2, S=2048, D=2048, H=16, KV=8, HD=128,
YL=256, YD=1024.

Sharding: 2-way batch DP x 4-way head TP.  Core c handles batch c//4 and
head group g=c%4 (q heads 4g..4g+3, kv heads 2g..2g+1, y-heads
(4g%8)..(4g%8)+3).  wo is row-sharded; the 4 partial outputs per batch are
summed on the host.  The q/k/ky layernorms normalize over the *full* flat
head dim, so each core computes partial (sum, sumsq) stats and three small
in-kernel AllReduces (groups [[0..3],[4..7]]) produce the full-row moments.

v2 engine-balance notes (vs the v1 baseline):
- PE only does real GEMMs: LN stats moved to Pool partition_all_reduce,
  softmax partition-reduction to Pool, denominator broadcast comes free
  from the all-reduce, V is projected directly transposed (lhsT=x chunk)
  so the per-chunk PE transposes disappear, and wo reads outT straight
  from SBUF with the output DMA'd directly from PSUM (no Act copy).
- wo for query-block 0 is interleaved into the (Act/exp-bound) attention
  pass of query-block 1 to fill PE gaps.
- tanh(gate) is folded into wvy columns on the host, removing all gate
  handling from the device tails.
- DMA queue assignment avoids head-of-line blocking: wq/x strips go on
  the SP queue first, wk/wv on the Act queue, y-side on the DVE queue,
  stat rows and consts on the Pool queue.
"""
import sys

sys.path.insert(0, "/opt/trn_rl_repo")

import numpy as np

import concourse.bass as bass  # noqa: F401
import concourse.tile as tile
from concourse import bacc, bass_isa, mybir
from concourse import bass_utils

BF16 = mybir.dt.bfloat16
DT16 = mybir.dt.float16
F32 = mybir.dt.float32
NP16 = np.float16

B, S, D, H, KV, YL, YD, HD = 2, 2048, 2048, 16, 8, 256, 1024, 128
N_CORES, TP = 8, 4
HPC, KVPC, YHPC = 4, 2, 4          # q / kv / y heads per core
QW, KW, YW = HPC * HD, KVPC * HD, YHPC * HD   # 512, 256, 512 output cols
NDC, NYC = D // 128, YD // 128     # contraction chunks: 16, 8
NSB, SB = 4, 512                   # seq blocks for projections
NQB, QB = 2, 1024                  # query blocks for attention
NKC = S // 128                     # 16 key chunks (self)
NYKC = YL // 128                   # 2 key chunks (cross)
NST = S // 128                     # 16 seq tiles for wo
EPS_QK, EPS_KY = 1e-5, 1e-6
NEG = -1.0e30

_RUNNER = None
_EXEC = None


def _build_program(use_cc=True):
    nc = bacc.Bacc("TRN2", target_bir_lowering=False, debug=False,
                   num_devices=N_CORES if use_cc else 1)

    def din(name, shape, dt=DT16):
        return nc.dram_tensor(name, shape, dt, kind="ExternalInput")

    t = dict(
        xT=din("xT", [D, S]),
        yT=din("yT", [YD, YL]),
        wq=din("wq", [D, QW]),
        wk=din("wk", [D, KW]),
        wv=din("wv", [D, KW]),
        wky=din("wky", [YD, YW]),
        wvy=din("wvy", [YD, YW]),
        wo=din("wo", [QW, D]),
        CC=din("CC", [128, S]),
        SSp=din("SSp", [128, S]),
        swapP=din("swapP", [128, 128]),
        qgc=din("qgc", [128, HPC], F32),
        kgc=din("kgc", [128, KVPC], F32),
        kygc=din("kygc", [128, YHPC], F32),
        qb=din("qb", [128, HPC], F32),
        kb=din("kb", [128, KVPC], F32),
        kyb=din("kyb", [128, YHPC], F32),
        xmask=din("xmask", [128, NKC], F32),
        ymask=din("ymask", [128, NYKC], F32),
        out=nc.dram_tensor("out", [S, D], DT16, kind="ExternalOutput"),
        kin=nc.dram_tensor("kin", [2 * NSB, SB], F32),
        kout=nc.dram_tensor("kout", [2 * NSB, SB], F32),
        kyin=nc.dram_tensor("kyin", [2, YL], F32),
        kyout=nc.dram_tensor("kyout", [2, YL], F32),
        qin=nc.dram_tensor("qin", [2, S], F32),
        qout=nc.dram_tensor("qout", [2, S], F32),
        lnr=nc.dram_tensor("lnr", [6, S], DT16),
        groups=[[0, 1, 2, 3], [4, 5, 6, 7]],
        use_cc=use_cc,
    )

    with tile.TileContext(nc) as tc:
        _emit(nc, tc, t)
    nc.compile()
    return nc


def _emit(nc, tc, t):
    AF = mybir.ActivationFunctionType
    Alu = mybir.AluOpType
    RO = bass_isa.ReduceOp

    def strip_load(dst, src_ap, nchunk, q=None):
        eng = q or nc.sync
        for s0 in range(0, nchunk, 2):
            eng.dma_start(dst[:, s0:s0 + 2, :], src_ap[:, s0:s0 + 2, :])

    cm_consts = tc.tile_pool(name="consts", bufs=1)
    consts = cm_consts.__enter__()

    # ---------------- phase-1 weight / input loads ----------------
    # wq + x(sb0) strips first on the SP queue so the first q matmul can
    # start as soon as its chunk lands; bulk k/v/y weights on other queues.
    cm_raw = tc.tile_pool(name="p_raw", bufs=1)
    p_raw = cm_raw.__enter__()
    cm_ph1 = tc.tile_pool(name="p_ph1", bufs=1)
    p_ph1 = cm_ph1.__enter__()
    cm_x = tc.tile_pool(name="p_x", bufs=3)
    p_x = cm_x.__enter__()

    xT_r = t["xT"].ap().rearrange("(c p) s -> p c s", p=128)
    wq_r = t["wq"].ap().rearrange("(c p) m -> p c m", p=128)
    wq_sb = p_ph1.tile([128, NDC, QW], DT16, tag="wq", name="wq")
    HC = NDC // 2

    def load_half(sb, half):
        """one x half-tile (8 chunks) on the SP queue."""
        tl = p_x.tile([128, HC, SB], DT16, tag="xtb", name="xtb")
        col = xT_r[:, :, sb * SB:(sb + 1) * SB]
        for s0, w in [(0, 4), (4, 4)]:
            nc.sync.dma_start(tl[:, s0:s0 + w, :],
                              col[:, half * HC + s0:half * HC + s0 + w, :])
        return tl

    # sb0: fine-grained wq/x strips interleaved so the first matmul can
    # start after the first two small transfers.
    lo0 = p_x.tile([128, HC, SB], DT16, tag="xtb", name="xtb")
    hi0 = p_x.tile([128, HC, SB], DT16, tag="xtb", name="xtb")
    col0 = xT_r[:, :, 0:SB]
    for s0, w in [(0, 1), (1, 1), (2, 2), (4, 4), (8, 8)]:
        nc.sync.dma_start(wq_sb[:, s0:s0 + w, :], wq_r[:, s0:s0 + w, :])
        if s0 < HC:
            dst = lo0 if s0 + w <= HC else None
            if dst is None:
                nc.sync.dma_start(lo0[:, s0:HC, :], col0[:, s0:HC, :])
                nc.sync.dma_start(hi0[:, 0:s0 + w - HC, :],
                                  col0[:, HC:s0 + w, :])
            else:
                nc.sync.dma_start(dst[:, s0:s0 + w, :], col0[:, s0:s0 + w, :])
        else:
            nc.sync.dma_start(hi0[:, s0 - HC:s0 - HC + w, :],
                              col0[:, s0:s0 + w, :])
    xtb0 = [lo0, hi0]

    wk_sb = p_ph1.tile([128, NDC, KW], DT16, tag="wk", name="wk")
    wv_sb = p_ph1.tile([128, NDC, KW], DT16, tag="wv", name="wv")
    yt = p_ph1.tile([128, NYC, YL], DT16, tag="yt", name="yt")
    wky_sb = p_ph1.tile([128, NYC, YW], DT16, tag="wky", name="wky")
    wvy_sb = p_ph1.tile([128, NYC, YW], DT16, tag="wvy", name="wvy")
    wo_sb = consts.tile([128, HPC, D], DT16, tag="wo", name="wo")

    def emit_bulk_loads():
        """k/v/y/wo weights on the Act queue; deferred into the q loop so
        they don't contend with the critical wq/x transfers at startup."""
        nc.scalar.dma_start(wk_sb[:, :, :],
                            t["wk"].ap().rearrange("(c p) m -> p c m", p=128))
        nc.scalar.dma_start(wv_sb[:, :, :],
                            t["wv"].ap().rearrange("(c p) m -> p c m", p=128))
        nc.scalar.dma_start(yt[:, :, :],
                            t["yT"].ap().rearrange("(c p) s -> p c s", p=128))
        nc.scalar.dma_start(wky_sb[:, :, :],
                            t["wky"].ap().rearrange("(c p) m -> p c m", p=128))
        nc.scalar.dma_start(wvy_sb[:, :, :],
                            t["wvy"].ap().rearrange("(c p) m -> p c m", p=128))
        nc.scalar.dma_start(wo_sb[:, :, :],
                            t["wo"].ap().rearrange("(c p) m -> p c m", p=128))

    # ---------------- constants / small inputs (Pool queue) ----------------
    swp = consts.tile([128, 128], DT16, tag="swp", name="swp")
    nc.scalar.dma_start(swp[:, :], t["swapP"].ap())
    cc = consts.tile([128, S], DT16, tag="cc", name="cc")
    nc.scalar.dma_start(cc[:, :], t["CC"].ap())
    ssp = consts.tile([128, S], DT16, tag="ssp", name="ssp")
    nc.scalar.dma_start(ssp[:, :], t["SSp"].ap())
    qg_sb = consts.tile([128, HPC], F32, tag="qgc", name="qgc")
    nc.scalar.dma_start(qg_sb[:, :], t["qgc"].ap())
    kg_sb = consts.tile([128, KVPC], F32, tag="kgc", name="kgc")
    nc.scalar.dma_start(kg_sb[:, :], t["kgc"].ap())
    kyg_sb = consts.tile([128, YHPC], F32, tag="kygc", name="kygc")
    nc.scalar.dma_start(kyg_sb[:, :], t["kygc"].ap())
    qb_sb = consts.tile([128, HPC], F32, tag="qb", name="qb")
    nc.scalar.dma_start(qb_sb[:, :], t["qb"].ap())
    kb_sb = consts.tile([128, KVPC], F32, tag="kb", name="kb")
    nc.scalar.dma_start(kb_sb[:, :], t["kb"].ap())
    kyb_sb = consts.tile([128, YHPC], F32, tag="kyb", name="kyb")
    nc.scalar.dma_start(kyb_sb[:, :], t["kyb"].ap())
    xm_sb = consts.tile([128, NKC], F32, tag="xm", name="xm")
    nc.scalar.dma_start(xm_sb[:, :], t["xmask"].ap())
    ym_sb = consts.tile([128, NYKC], F32, tag="ym", name="ym")
    nc.scalar.dma_start(ym_sb[:, :], t["ymask"].ap())

    # ---------------- projection-phase pools ----------------
    cm_wsq = tc.tile_pool(name="w_sq", bufs=1)
    w_sq = cm_wsq.__enter__()
    cm_war = tc.tile_pool(name="w_ar", bufs=1)
    w_ar = cm_war.__enter__()
    cm_psA = tc.tile_pool(name="pp_projA", bufs=4, space="PSUM")
    pp_proj = cm_psA.__enter__()

    kraw = [p_raw.tile([128, S], DT16, tag=f"kraw{i}", name=f"kraw{i}")
            for i in range(KVPC)]
    qraw = [p_raw.tile([128, S], DT16, tag=f"qraw{i}", name=f"qraw{i}")
            for i in range(HPC)]
    ykraw = [p_raw.tile([128, YL], DT16, tag=f"ykraw{i}", name=f"ykraw{i}")
             for i in range(YHPC)]

    cm_qkv = tc.tile_pool(name="p_qkv", bufs=1, side="right")
    p_qkv = cm_qkv.__enter__()
    vnat = p_qkv.tile([128, NKC, KW], DT16, tag="vnat", name="vnat")
    yvnat = p_qkv.tile([128, NYKC, YW], DT16, tag="yvnat", name="yvnat")

    def stats_block(raws, sb0, blk, dram, sbi, rowpair=False):
        """Partition+head sums of raws[:, sb0:sb0+blk] and their squares ->
        dram rows 0/1 via Pool all-reduce.  DVE does the cheap 16-bit
        pairwise head adds; Pool upcasts to f32 across partitions."""
        n = len(raws)
        ss = w_sq.tile([128, SB], DT16, tag="s1", name="s1")
        nc.vector.tensor_add(ss[:, :blk], raws[0][:, sb0:sb0 + blk],
                             raws[1][:, sb0:sb0 + blk])
        if n == 4:
            s2 = w_sq.tile([128, SB], DT16, tag="s2", name="s2")
            nc.vector.tensor_add(s2[:, :blk], raws[2][:, sb0:sb0 + blk],
                                 raws[3][:, sb0:sb0 + blk])
            nc.vector.tensor_add(ss[:, :blk], ss[:, :blk], s2[:, :blk])
        sq = w_sq.tile([128, SB], BF16, tag="sqa", name="sqa")
        nc.vector.tensor_mul(sq[:, :blk], raws[0][:, sb0:sb0 + blk],
                             raws[0][:, sb0:sb0 + blk])
        sqb = w_sq.tile([128, SB], BF16, tag="sqb", name="sqb")
        for i in range(1, n):
            nc.vector.tensor_mul(sqb[:, :blk], raws[i][:, sb0:sb0 + blk],
                                 raws[i][:, sb0:sb0 + blk])
            nc.vector.tensor_add(sq[:, :blk], sq[:, :blk], sqb[:, :blk])
        ar_s = w_ar.tile([128, SB], F32, tag="ars", name="ars")
        nc.gpsimd.partition_all_reduce(ar_s[:, :blk], ss[:, :blk], 128, RO.add)
        ar_q = w_ar.tile([128, SB], F32, tag="arq", name="arq")
        nc.gpsimd.partition_all_reduce(ar_q[:, :blk], sq[:, :blk], 128, RO.add)
        if rowpair:
            nc.gpsimd.dma_start(dram.ap()[2 * sbi:2 * sbi + 1, 0:blk],
                                ar_s[0:1, :blk])
            nc.gpsimd.dma_start(dram.ap()[2 * sbi + 1:2 * sbi + 2, 0:blk],
                                ar_q[0:1, :blk])
        else:
            nc.gpsimd.dma_start(dram.ap()[0:1, sbi * SB:sbi * SB + blk],
                                ar_s[0:1, :blk])
            nc.gpsimd.dma_start(dram.ap()[1:2, sbi * SB:sbi * SB + blk],
                                ar_q[0:1, :blk])

    def proj_col(w_sb, nchunk, src, col0, blk, ps):
        for c in range(nchunk):
            s = (src[c // HC][:, c % HC, :] if isinstance(src, (tuple, list))
                 else src[:, c, :])
            nc.tensor.matmul(ps[:, :blk], w_sb[:, c, col0:col0 + 128],
                             s, start=(c == 0), stop=(c == nchunk - 1))

    # ============ moments / LN helpers ============
    cm_rm = tc.tile_pool(name="rows_m", bufs=1, side="right")
    rows_m = cm_rm.__enter__()
    cm_wln = tc.tile_pool(name="w_ln", bufs=1, side="right")
    w_ln = cm_wln.__enter__()
    cm_wln2 = tc.tile_pool(name="w_ln2", bufs=2, side="right")
    w_ln2 = cm_wln2.__enter__()
    cm_wsw = tc.tile_pool(name="w_sw", bufs=1, side="right")
    w_sw = cm_wsw.__enter__()

    def moments(srct, n, inv_scale, eps, length, r_rstd, r_nmr,
                p0=0, NP=128, packed_sb=False):
        """sum/sumsq rows of srct -> rstd and -mu*rstd rows of lnr.
        Partition-parallel on [NP, J] tiles; p0/NP select a seq slice
        (seq s lives at partition s//J, col s%J) for pipelined emission."""
        J = length // 128
        def rd(row):
            tile_ = rows_m.tile([128, 16], F32, tag=f"m{row}", name=f"m{row}")
            if packed_sb:
                off = (2 * (p0 // 32) + row) * SB
            else:
                off = row * length + p0 * J
            ap = bass.AP(tensor=srct.ap().tensor, offset=off,
                         ap=[[J, NP], [1, J]])
            nc.sync.dma_start(tile_[:NP, :J], ap)
            return tile_
        a = rd(0)
        nc.vector.tensor_scalar_mul(a[:NP, :J], a[:NP, :J], inv_scale / n)
        b = rd(1)
        nc.vector.tensor_scalar_mul(b[:NP, :J], b[:NP, :J], inv_scale / n)
        c = rows_m.tile([128, 16], F32, tag="mc", name="mc")
        nc.vector.tensor_mul(c[:NP, :J], a[:NP, :J], a[:NP, :J])
        nc.vector.tensor_tensor(b[:NP, :J], b[:NP, :J], c[:NP, :J],
                                Alu.subtract)
        nc.vector.tensor_scalar_add(b[:NP, :J], b[:NP, :J], eps)
        nc.scalar.activation(c[:NP, :J], b[:NP, :J], AF.Sqrt)
        nc.vector.reciprocal(c[:NP, :J], c[:NP, :J])
        d = rows_m.tile([128, 16], F32, tag="md", name="md")
        nc.vector.tensor_mul(d[:NP, :J], c[:NP, :J], c[:NP, :J])
        nc.vector.tensor_mul(d[:NP, :J], d[:NP, :J], b[:NP, :J])
        nc.vector.tensor_scalar(out=d[:NP, :J], in0=d[:NP, :J],
                                scalar1=-0.5, scalar2=1.5,
                                op0=Alu.mult, op1=Alu.add)
        nc.vector.tensor_mul(c[:NP, :J], c[:NP, :J], d[:NP, :J])
        nc.vector.tensor_mul(a[:NP, :J], a[:NP, :J], c[:NP, :J])
        nc.vector.tensor_scalar_mul(a[:NP, :J], a[:NP, :J], -1.0)
        ch = rows_m.tile([128, 16], DT16, tag="mch", name="mch")
        nc.vector.tensor_copy(ch[:NP, :J], c[:NP, :J])
        ah = rows_m.tile([128, 16], DT16, tag="mah", name="mah")
        nc.vector.tensor_copy(ah[:NP, :J], a[:NP, :J])
        out_r = bass.AP(tensor=t["lnr"].ap().tensor,
                        offset=r_rstd * S + p0 * J, ap=[[J, NP], [1, J]])
        nc.sync.dma_start(out_r, ch[:NP, :J])
        out_n = bass.AP(tensor=t["lnr"].ap().tensor,
                        offset=r_nmr * S + p0 * J, ap=[[J, NP], [1, J]])
        nc.sync.dma_start(out_n, ah[:NP, :J])

    def dma_bcast(dst, row, length):
        src_ap = bass.AP(tensor=t["lnr"].ap().tensor, offset=row * S,
                         ap=[[0, 128], [1, length]])
        nc.sync.dma_start(dst[:, :length], src_ap)

    def ln_prep(key, r_rstd, r_nmr, length):
        rg = w_ln.tile([128, length], DT16, tag=f"rg{key}", name=f"rg{key}")
        dma_bcast(rg, r_rstd, length)
        ng = w_ln.tile([128, length], DT16, tag=f"ng{key}", name=f"ng{key}")
        dma_bcast(ng, r_nmr, length)
        return rg, ng

    def ln_pre(raw, i, rg, ng, g_cols, b_cols, length):
        """DVE-only half of LN: t1 = (raw*rg + ng)*g + b."""
        t1 = w_ln2.tile([128, length], DT16, tag="lnt1", name="lnt1")
        nc.vector.tensor_mul(t1[:, :length], raw[:, :length], rg[:, :length])
        nc.vector.tensor_add(t1[:, :length], t1[:, :length], ng[:, :length])
        nc.vector.tensor_scalar(out=t1[:, :length], in0=t1[:, :length],
                                scalar1=g_cols[:, i:i + 1],
                                scalar2=b_cols[:, i:i + 1],
                                op0=Alu.mult, op1=Alu.add)
        return t1

    def ln_post(t1, i, length, rope, fin_tag, mkps, sw_dve=False):
        """Rope half: PE half-swap + cos/sin muls.  Emitted a phase after
        ln_pre so the swap matmuls never park in the PE wait queue."""
        fin = p_qkv.tile([128, length], DT16, tag=f"{fin_tag}{i}",
                         name=f"{fin_tag}{i}")
        if not rope:
            nc.vector.tensor_copy(fin[:, :length], t1[:, :length])
            return fin
        sw = w_sw.tile([128, length], DT16, tag="swap", name="swap")
        for j in range(0, length, SB):
            ps = mkps()
            nc.tensor.matmul(ps[:, :SB], swp[:, :], t1[:, j:j + SB],
                             start=True, stop=True)
            if sw_dve:
                nc.vector.tensor_copy(sw[:, j:j + SB], ps[:, :SB])
            else:
                nc.scalar.activation(sw[:, j:j + SB], ps[:, :SB], AF.Copy)
        nc.vector.tensor_mul(t1[:, :length], t1[:, :length], cc[:, :length])
        nc.vector.tensor_mul(sw[:, :length], sw[:, :length], ssp[:, :length])
        nc.vector.tensor_add(fin[:, :length], t1[:, :length], sw[:, :length])
        return fin

    def mk_proj():
        return pp_proj.tile([128, SB], F32, tag="proj", name="proj")

    # ============ q projections + stats + AR-q ============
    nxt_x = [xtb0]
    for sb in range(NSB):
        xtb = list(nxt_x.pop(0))
        for i in range(HPC):
            ps = mk_proj()
            proj_col(wq_sb, NDC, xtb, i * 128, SB, ps)
            nc.scalar.activation(qraw[i][:, sb * SB:(sb + 1) * SB],
                                 ps[:, :], AF.Copy)
            if sb + 1 < NSB:
                if i == 0:
                    nxt_x.append([load_half(sb + 1, 0), None])
                elif i == 2:
                    nxt_x[0][1] = load_half(sb + 1, 1)
            if sb == 1 and i == 0:
                emit_bulk_loads()
        stats_block(qraw, sb * SB, SB, t["qin"], sb)

    if t["use_cc"]:
        nc.gpsimd.collective_compute(
            "AllReduce", Alu.add, replica_groups=t["groups"],
            ins=[t["qin"].ap().opt()], outs=[t["qout"].ap().opt()])

    moments(t["qout"] if t["use_cc"] else t["qin"],
            H * HD, 1.0, EPS_QK, S, 0, 1)
    rgq, ngq = ln_prep("q", 0, 1, S)

    # ============ k + v projections, LN-q interleaved per sb ============
    QT = [None] * HPC
    qt1 = [None] * HPC
    rgk = w_ln.tile([128, S], DT16, tag="rgk", name="rgk")
    ngk = w_ln.tile([128, S], DT16, tag="ngk", name="ngk")
    kt1 = [w_ln.tile([128, S], DT16, tag=f"kt1{i}", name=f"kt1{i}")
           for i in range(KVPC)]
    nxt_x = [[load_half(0, 0), load_half(0, 1)]]
    for sb in range(NSB):
        xtb = list(nxt_x.pop(0))
        for i in range(KVPC):
            ps = mk_proj()
            proj_col(wk_sb, NDC, xtb, i * 128, SB, ps)
            if sb + 1 < NSB:
                if i == 0:
                    nxt_x.append([load_half(sb + 1, 0), None])
                else:
                    nxt_x[0][1] = load_half(sb + 1, 1)
            nc.scalar.activation(kraw[i][:, sb * SB:(sb + 1) * SB],
                                 ps[:, :], AF.Copy)
        stats_block(kraw, sb * SB, SB, t["kin"], sb, rowpair=True)
        # v directly transposed: lhsT = x chunk (seq cols), moving = wv
        for s in range(4):
            psf = mk_proj()
            ps = psf[:, :KW]
            for c in range(NDC):
                xs = xtb[c // HC][:, c % HC, s * 128:(s + 1) * 128]
                nc.tensor.matmul(ps[:, :], xs, wv_sb[:, c, :],
                                 start=(c == 0), stop=(c == NDC - 1))
            nc.scalar.activation(vnat[:, sb * 4 + s, :], ps[:, :], AF.Copy)
        kin_sl = t["kin"].ap()[2 * sb:2 * sb + 2, :]
        kout_sl = t["kout"].ap()[2 * sb:2 * sb + 2, :]
        if t["use_cc"]:
            nc.gpsimd.collective_compute(
                "AllReduce", Alu.add, replica_groups=t["groups"],
                ins=[kin_sl.opt()], outs=[kout_sl.opt()])
        qt1[sb] = ln_pre(qraw[sb], sb, rgq, ngq, qg_sb, qb_sb, S)
        if sb > 0:
            QT[sb - 1] = ln_post(qt1[sb - 1], sb - 1, S, True, "QT", mk_proj)
        moments(t["kout"] if t["use_cc"] else t["kin"],
                KV * HD, 1.0, EPS_QK, S, 2, 3, p0=sb * 32, NP=32,
                packed_sb=True)
        # LN-k slice for this sb: broadcast the fresh lnr rows and apply
        sl = slice(sb * SB, (sb + 1) * SB)
        for dst, row in ((rgk, 2), (ngk, 3)):
            src_ap = bass.AP(tensor=t["lnr"].ap().tensor,
                             offset=row * S + sb * SB,
                             ap=[[0, 128], [1, SB]])
            nc.sync.dma_start(dst[:, sl], src_ap)
        for i in range(KVPC):
            nc.vector.tensor_mul(kt1[i][:, sl], kraw[i][:, sl], rgk[:, sl])
            nc.vector.tensor_add(kt1[i][:, sl], kt1[i][:, sl], ngk[:, sl])
            nc.vector.tensor_scalar(out=kt1[i][:, sl], in0=kt1[i][:, sl],
                                    scalar1=kg_sb[:, i:i + 1],
                                    scalar2=kb_sb[:, i:i + 1],
                                    op0=Alu.mult, op1=Alu.add)

    # ============ y projections ============
    for i in range(YHPC):
        psf = mk_proj()
        ps = psf[:, :YL]
        proj_col(wky_sb, NYC, yt, i * 128, YL, ps)
        nc.scalar.activation(ykraw[i][:, :], ps[:, :YL], AF.Copy)
    stats_block(ykraw, 0, YL, t["kyin"], 0)
    for s in range(NYKC):
        ps = mk_proj()
        for c in range(NYC):
            nc.tensor.matmul(ps[:, :], yt[:, c, s * 128:(s + 1) * 128],
                             wvy_sb[:, c, :], start=(c == 0),
                             stop=(c == NYC - 1))
        nc.scalar.activation(yvnat[:, s, :], ps[:, :], AF.Copy)

    if t["use_cc"]:
        nc.gpsimd.collective_compute(
            "AllReduce", Alu.add, replica_groups=t["groups"],
            ins=[t["kyin"].ap().opt()], outs=[t["kyout"].ap().opt()])

    QT[3] = ln_post(qt1[3], 3, S, True, "QT", mk_proj)
    moments(t["kyout"] if t["use_cc"] else t["kyin"],
            KV * HD, 0.5, EPS_KY, YL, 4, 5)
    rgy, ngy = ln_prep("y", 4, 5, YL)
    YKT = [ln_post(ln_pre(ykraw[i], i, rgy, ngy, kyg_sb, kyb_sb, YL),
                   i, YL, False, "YKT", None) for i in range(YHPC)]

    cm_war.__exit__(None, None, None)
    cm_wsq.__exit__(None, None, None)
    cm_x.__exit__(None, None, None)
    cm_ph1.__exit__(None, None, None)
    cm_psA.__exit__(None, None, None)

    # ============ attention + interleaved wo ============
    cm_out = tc.tile_pool(name="p_out", bufs=1)
    p_out = cm_out.__enter__()
    outT = [p_out.tile([128, S], DT16, tag=f"outT{h}", name=f"outT{h}")
            for h in range(HPC)]
    cm_wat = tc.tile_pool(name="w_at", bufs=4)
    w_at = cm_wat.__enter__()
    cm_pt = tc.tile_pool(name="w_pt", bufs=8)
    w_pt = cm_pt.__enter__()
    cm_den = tc.tile_pool(name="w_den", bufs=3)
    w_den = cm_den.__enter__()
    cm_ob = tc.tile_pool(name="w_ob", bufs=8)
    w_ob = cm_ob.__enter__()
    cm_pvb = tc.tile_pool(name="w_pvb", bufs=1)
    w_pvb = cm_pvb.__enter__()

    cm_sc = tc.tile_pool(name="pp_sc", bufs=2, space="PSUM")
    cm_pv = tc.tile_pool(name="pp_pv", bufs=2, space="PSUM")
    pp_sc = cm_sc.__enter__()
    pp_pv = cm_pv.__enter__()

    def mk_sc():
        return pp_sc.tile([128, QB], F32, tag="sc", name="sc")


    KT = [None] * KVPC
    KT[0] = ln_post(kt1[0], 0, S, True, "KT", mk_sc)

    def attend(h, qb_i, KT_h, v_tile, vcol, nkc, mask_sb, early_free=False):
        q0 = qb_i * QB
        pv = pp_pv.tile([128, QB], F32, tag="pv", name="pv")
        acc = w_at.tile([128, QB], DT16, tag="acc", name="acc")
        scs = []

        def sc_mm(c):
            sc = mk_sc()
            for j in range(0, QB, SB):
                nc.tensor.matmul(sc[:, j:j + SB],
                                 KT_h[:, c * 128:(c + 1) * 128],
                                 QT[h][:, q0 + j:q0 + j + SB],
                                 start=True, stop=True)
            scs.append(sc)

        sc_mm(0)
        for c in range(nkc):
            pt = w_pt.tile([128, QB], DT16, tag="ptile", name="ptile")
            nc.scalar.activation(pt[:, :], scs[c][:, :], AF.Exp,
                                 bias=mask_sb[:, c:c + 1])
            if c + 1 < nkc:
                sc_mm(c + 1)
            for j in range(0, QB, SB):
                nc.tensor.matmul(pv[:, j:j + SB],
                                 v_tile[:, c, vcol:vcol + 128],
                                 pt[:, j:j + SB],
                                 start=(c == 0), stop=(c == nkc - 1))
            if c == 0:
                nc.vector.tensor_copy(acc[:, :], pt[:, :])
            else:
                nc.vector.tensor_add(acc[:, :], acc[:, :], pt[:, :])
        src_pv = pv
        if early_free:
            # free the pv PSUM before the den chain so the next head's pv
            # allocation isn't blocked ~4us by allreduce+recip
            pvb = w_pvb.tile([128, QB], F32, tag="pvb", name="pvb")
            nc.vector.tensor_copy(pvb[:, :], pv[:, :])
            src_pv = pvb
        den = w_den.tile([128, QB], F32, tag="den", name="den")
        nc.gpsimd.partition_all_reduce(den[:, :], acc[:, :], 128, RO.add)
        r = w_den.tile([128, QB], F32, tag="rcp", name="rcp")
        nc.vector.reciprocal(r[:, :], den[:, :])
        o = w_at.tile([128, QB], DT16, tag="oattn", name="oattn")
        nc.vector.tensor_mul(o[:, :], src_pv[:, :], r[:, :])
        return o

    def wo_chunk(st, j, alt=0):
        pool = pp_sc if alt % 2 else pp_pv
        psf = pool.tile([128, QB], F32, tag=("sc" if alt % 2 else "pv"),
                        name="wops")
        ps = psf[:, :SB]
        for dc in range(HPC):
            nc.tensor.matmul(ps[:, :], outT[dc][:, st * 128:(st + 1) * 128],
                             wo_sb[:, dc, j:j + SB],
                             start=(dc == 0), stop=(dc == HPC - 1))
        ob = w_ob.tile([128, SB], DT16, tag="wob", name="wob")
        if alt % 2:
            nc.scalar.activation(ob[:, :], ps[:, :], AF.Copy)
        else:
            nc.vector.tensor_copy(ob[:, :], ps[:, :])
        nc.sync.dma_start(
            t["out"].ap()[st * 128:(st + 1) * 128, j:j + SB], ob[:, :])

    pending = []   # (st, j) wo chunks ready to emit

    def emit_wo(k, alternate=False):
        i = 0
        for _ in range(min(k, len(pending))):
            st, j = pending.pop(0)
            wo_chunk(st, j, alt=(i if alternate else 0))
            i += 1

    for qb_i in range(NQB):
        for h in range(HPC):
            if qb_i == 0 and h == 0:
                # self first on the very first iteration: the YKT
                # moments/broadcast DMA chain is still in flight then
                o_self = attend(h, qb_i, KT[0], vnat, 0, NKC, xm_sb,
                                early_free=True)
                o_y = attend(h, qb_i, YKT[0], yvnat, 0, NYKC, ym_sb)
            else:
                o_y = attend(h, qb_i, YKT[h], yvnat, h * 128, NYKC, ym_sb)
                o_self = attend(h, qb_i, KT[h // 2], vnat, (h // 2) * 128,
                                NKC, xm_sb)
            nc.vector.tensor_add(outT[h][:, qb_i * QB:(qb_i + 1) * QB],
                                 o_self[:, :], o_y[:, :])
            if qb_i == 0 and h == 0:
                KT[1] = ln_post(kt1[1], 1, S, True, "KT", mk_sc)
            emit_wo(8)
        if qb_i == 0:
            pending.extend([(st, j) for st in range(8)
                            for j in range(0, D, SB)])
    pending.extend([(st, j) for st in range(8, 16)
                    for j in range(0, D, SB)])
    emit_wo(len(pending), alternate=True)

    cm_pv.__exit__(None, None, None)
    cm_sc.__exit__(None, None, None)
    cm_pvb.__exit__(None, None, None)
    cm_ob.__exit__(None, None, None)
    cm_den.__exit__(None, None, None)
    cm_pt.__exit__(None, None, None)
    cm_wat.__exit__(None, None, None)
    cm_out.__exit__(None, None, None)
    cm_wsw.__exit__(None, None, None)
    cm_wln2.__exit__(None, None, None)
    cm_wln.__exit__(None, None, None)
    cm_rm.__exit__(None, None, None)
    cm_qkv.__exit__(None, None, None)
    cm_raw.__exit__(None, None, None)
    cm_consts.__exit__(None, None, None)


def _perm_cols(ncols):
    p = np.arange(ncols).reshape(-1, HD)
    return np.concatenate([p[:, 0::2], p[:, 1::2]], axis=1).reshape(-1)


def _prep_core_inputs(inputs, core):
    b, g = core // TP, core % TP
    f32 = np.float32
    x = np.asarray(inputs["x"], f32)
    y = np.asarray(inputs["y"], f32)

    qcols = np.arange(g * QW, (g + 1) * QW)
    kcols = np.arange(g * KW, (g + 1) * KW)
    y0 = (4 * g % 8) * HD
    ycols = np.arange(y0, y0 + YW)
    qperm = qcols[_perm_cols(QW)]
    kperm = kcols[_perm_cols(KW)]
    yperm = ycols[_perm_cols(YW)]

    scale = 1.0 / np.sqrt(HD)
    qg = (np.asarray(inputs["q_norm_g"], f32) * scale)[qperm]
    qb = (np.asarray(inputs["q_norm_b"], f32) * scale)[qperm]
    kg = np.asarray(inputs["k_norm_g"], f32)[kperm]
    kb = np.asarray(inputs["k_norm_b"], f32)[kperm]
    kyg = np.asarray(inputs["ky_norm_g"], f32)[yperm]
    kyb = np.asarray(inputs["ky_norm_b"], f32)[yperm]

    cos = np.asarray(inputs["freqs_cos"], f32)[b].T
    sin = np.asarray(inputs["freqs_sin"], f32)[b].T
    CCm = np.concatenate([cos, cos], 0)
    SSm = np.concatenate([-sin, sin], 0)
    swapP = np.zeros((128, 128), f32)
    swapP[np.arange(128), (np.arange(128) + 64) % 128] = 1.0

    xm = np.where(np.asarray(inputs["x_mask"][b]), 0.0, NEG).astype(f32)
    ym = np.where(np.asarray(inputs["y_mask"][b]), 0.0, NEG).astype(f32)

    # fold tanh(gate) per y-head into the wvy columns (gated cross-attn)
    tgv = np.tanh(np.asarray(inputs["gate"], f32)[4 * g:4 * g + 4])
    wvy = np.asarray(inputs["wv_y"], f32)[:, ycols].copy()
    for i in range(YHPC):
        wvy[:, i * HD:(i + 1) * HD] *= tgv[i]

    bf = lambda a: np.ascontiguousarray(a).astype(NP16)
    return {
        "xT": bf(x[b].T), "yT": bf(y[b].T),
        "wq": bf(np.asarray(inputs["wq"], f32)[:, qperm]),
        "wk": bf(np.asarray(inputs["wk"], f32)[:, kperm]),
        "wv": bf(np.asarray(inputs["wv"], f32)[:, kcols]),
        "wky": bf(np.asarray(inputs["wk_y"], f32)[:, yperm]),
        "wvy": bf(wvy),
        "wo": bf(np.asarray(inputs["wo"], f32)[qcols, :]),
        "CC": bf(CCm), "SSp": bf(SSm), "swapP": bf(swapP),
        "qgc": np.ascontiguousarray(qg.reshape(HPC, HD).T).astype(f32),
        "kgc": np.ascontiguousarray(kg.reshape(KVPC, HD).T).astype(f32),
        "kygc": np.ascontiguousarray(kyg.reshape(YHPC, HD).T).astype(f32),
        "qb": np.ascontiguousarray(qb.reshape(HPC, HD).T).astype(f32),
        "kb": np.ascontiguousarray(kb.reshape(KVPC, HD).T).astype(f32),
        "kyb": np.ascontiguousarray(kyb.reshape(YHPC, HD).T).astype(f32),
        "xmask": np.ascontiguousarray(xm.reshape(NKC, 128).T).astype(f32),
        "ymask": np.ascontiguousarray(ym.reshape(NYKC, 128).T).astype(f32),
    }


def _get_runner():
    global _RUNNER
    if _RUNNER is None:
        _RUNNER = _build_program()
    return _RUNNER


def _get_exec():
    """Build (once) a cached jitted shard_map executable for the program."""
    global _EXEC
    if _EXEC is None:
        import jax
        from jax.experimental.shard_map import shard_map
        from jax.sharding import Mesh, NamedSharding, PartitionSpec

        nc = _get_runner()
        from concourse import bass2jax as b2j
        b2j.install_neuronx_cc_hook()

        pname = (nc.partition_id_tensor.name
                 if nc.partition_id_tensor else None)
        in_names, out_names, out_avals = [], [], []
        for alloc in nc.m.functions[0].allocations:
            if not isinstance(alloc, mybir.MemoryLocationSet):
                continue
            name = alloc.memorylocations[0].name
            if alloc.kind == "ExternalInput":
                if name != pname:
                    in_names.append(name)
            elif alloc.kind == "ExternalOutput":
                out_names.append(name)
                out_avals.append(jax.core.ShapedArray(
                    tuple(alloc.tensor_shape), mybir.dt.np(alloc.dtype)))
        n_params = len(in_names)
        all_in = list(in_names + out_names)
        if pname is not None:
            all_in.append(pname)
        all_in = tuple(all_in)
        donate = tuple(range(n_params, n_params + len(out_names)))

        def _body(*args):
            operands = list(args)
            if pname is not None:
                operands.append(b2j.partition_id_tensor())
            outs = b2j._bass_exec_p.bind(
                *operands, out_avals=tuple(out_avals), in_names=all_in,
                out_names=tuple(out_names),
                lowering_input_output_aliases=(),
                sim_require_finite=True, sim_require_nnan=True, nc=nc)
            return tuple(outs)

        devices = jax.devices()[:N_CORES]
        mesh = Mesh(np.asarray(devices), ("core",))
        nin = n_params + len(out_names)
        sharded = jax.jit(
            shard_map(_body, mesh=mesh,
                      in_specs=(PartitionSpec("core"),) * nin,
                      out_specs=(PartitionSpec("core"),) * len(out_names),
                      check_rep=False),
            donate_argnums=donate, keep_unused=True)
        shd = NamedSharding(mesh, PartitionSpec("core"))
        mk0 = [jax.jit(lambda a=a: __import__("jax.numpy", fromlist=["x"]
                                              ).zeros((N_CORES * a.shape[0],)
                                                      + a.shape[1:], a.dtype),
                       out_shardings=shd) for a in out_avals]
        _EXEC = (sharded, in_names, out_names, out_avals, shd, mk0)
    return _EXEC


def _concat_inputs(in_maps):
    sharded, in_names, out_names, out_avals, shd, mk0 = _get_exec()
    return [np.concatenate([np.asarray(in_maps[c][nm])
                            for c in range(N_CORES)], axis=0)
            for nm in in_names]


def _exec(concat_in, device_put=False):
    """Run once; returns {name: full concatenated np array}."""
    import jax
    sharded, in_names, out_names, out_avals, shd, mk0 = _get_exec()
    if device_put:
        concat_in = [jax.device_put(a, shd) for a in concat_in]
    outs = sharded(*concat_in, *[f() for f in mk0])
    return dict(zip(out_names, outs))


def run_on_cores(in_maps, trace=False):
    nc = _get_runner()
    return bass_utils.run_bass_kernel_spmd(
        nc, in_maps, core_ids=list(range(N_CORES)), trace=trace)


def kernel(**inputs):
    in_maps = [_prep_core_inputs(inputs, c) for c in range(N_CORES)]
    outs = _exec(_concat_inputs(in_maps))
    o = np.asarray(outs["out"]).reshape(N_CORES, S, D)
    out = np.zeros((B, S, D), np.float32)
    for c in range(N_CORES):
        out[c // TP] += o[c]
    return out
